# revision 17
# baseline (speedup 1.0000x reference)
"""CenterLoss (segment_reduce) Trainium2 kernel — fp8 two-stream design.

Strategy (data-parallel over N across 8 cores, ~65536 rows/core):
  Host prep: features cast to fp8 e4m3 (clip +-240).  Two HBM layouts:
      features_n  [128, T*257]  natural, host-swizzled so tile t partition p
                  holds row t*128+p as 256 fp8 values + a baked-in 1.0
                  (feeds the count column) -> contiguous group loads.
      features_ta/tb [128, R]   transposed halves (d on partitions) for the
                  pass-2 f.c matmuls.
  Pass 1: per 32-tile group one ~1MB load; batched one-hot for the whole
    group via a single broadcast is_equal; per tile matmul
    one_hot.T @ [f | 1] accumulating [8, 257] per-class sums+counts in
    PSUM (fp8 moving operand streams ~4 elem/cycle -> ~16us).
  ||f||^2: computed entirely in the transposed domain.  The resident
    ftA/ftB tiles are squared elementwise (ScalarE batched Square for
    most blocks, DVE tensor_tensor for FT2_DVE_BLOCKS of them to
    balance engines), then the PE reduces each squared [128d, 128n]
    chunk with a ones[128,1] rhs: f2 lands per-tile as [128,1] columns
    packed into per-block PSUM banks that pass-2 reads directly.  This
    beat ScalarE accum_out (~600ns/tile) and DVE tensor_scalar accum
    (~330ns/tile effective) by a wide margin.
  AllReduce the [8, 257] (tiny), compute centers, c2 (+BIG for empty
    classes), W = -2 centers.T as fp8 [128, 8] x2.
  Pass 2: per 64-tile block, fc = f.W accumulated into a [128, 512] PSUM
    bank on top of a rank-1 (K=1) matmul that pre-fills the bank with the
    per-class c2 row -> PSUM holds c2 - 2 f.c directly.  DVE min-reduces
    over classes straight out of PSUM and adds f2 from the f2 PSUM bank;
    one batched end-of-rep Sqrt with accum_out produces the per-core sum.
  Output: per-core scalar; host divides by N.  Loss rel err vs the f32
  reference: ~4e-3 (tolerance 2e-2).
"""

import numpy as np

from concourse import bass, bacc, mybir, tile
from concourse import bass_utils

F32 = mybir.dt.float32
F32R = mybir.dt.float32r
BF16 = mybir.dt.bfloat16
FP8 = mybir.dt.float8e4
OP = mybir.AluOpType
AFT = mybir.ActivationFunctionType

N_TOTAL = 524288
D = 256
C = 8
NCORES = 8
P = 128
BIG = 1.0e30
DW = D + 1  # 257: features + baked ones column

GROUP = 32       # tiles per staged load (~1MB)
BLOCK = 64       # tiles per PSUM bank / ft block (8192 rows)
FT_PREFETCH = 4  # ft double-buffer depth (= ft pool bufs)
SCALAR_OF_8 = 4   # (legacy knob for f2_mode="scalar"/"split")
PE_BLOCKS = 8     # blocks whose ||f||^2 = PE ones-matmul over squared fT
FT2_DVE_BLOCKS = 2  # of the PE blocks, how many get their fT squared on DVE
DVE_SQ_GROUPS = 0   # trailing natural groups whose batched square runs on DVE


def _issue_ft(nc, ftp, f_ta, f_tb, b):
    sl = slice(b * BLOCK * P, (b + 1) * BLOCK * P)
    ftA = ftp.tile([P, BLOCK * P], FP8, tag="ftA")
    ftB = ftp.tile([P, BLOCK * P], FP8, tag="ftB")
    nc.scalar.dma_start(ftA[:], f_ta.ap()[:, sl])
    nc.scalar.dma_start(ftB[:], f_tb.ap()[:, sl])
    return ftA, ftB


def build_nc(R: int, reps: int = 1, bf16_in: bool = False,
             f2_mode: str = "batch", p1mm: bool = True, p2: bool = True,
             nat_loads: bool = True, ft_loads: bool = True,
             pe_blocks: int = PE_BLOCKS, ft2_dve: int = FT2_DVE_BLOCKS,
             dve_sq_groups: int = DVE_SQ_GROUPS):
    """Build the SPMD bass program for R rows per core.

    f2_mode/p1mm/p2/nat_loads/ft_loads are timing-microbench knobs that
    carve the kernel down to a subset of its engine work; defaults build
    the full kernel.  f2_mode: split|scalar|dve|off.
    """
    assert R % (P * BLOCK) == 0
    T = R // P             # 128-row tiles per core
    nblk = T // BLOCK
    G = T // GROUP

    nc = bacc.Bacc(
        "TRN2", target_bir_lowering=False, debug=False, num_devices=NCORES
    )
    f_n = nc.dram_tensor("features_n", [P, T * DW], FP8, kind="ExternalInput")
    f_ta = nc.dram_tensor("features_ta", [P, R], FP8, kind="ExternalInput")
    f_tb = nc.dram_tensor("features_tb", [P, R], FP8, kind="ExternalInput")
    t_in = nc.dram_tensor("targets_f", [P, T], FP8, kind="ExternalInput")
    out_t = nc.dram_tensor("partial", [reps, 1], F32, kind="ExternalOutput")

    with tile.TileContext(nc) as tc:
        with (
            tc.tile_pool(name="const", bufs=1) as constp,
            tc.tile_pool(name="stage", bufs=3) as stagep,
            tc.tile_pool(name="oh", bufs=3) as ohp,
            tc.tile_pool(name="sq", bufs=2) as sqp,
            tc.tile_pool(name="sqv", bufs=2) as sqvp,
            tc.tile_pool(name="ft", bufs=FT_PREFETCH) as ftp,
            tc.tile_pool(name="ft2", bufs=2) as ft2p,
            tc.tile_pool(name="dist", bufs=2) as distp,
            tc.tile_pool(name="small", bufs=1) as smallp,
            tc.tile_pool(name="ps_acc", bufs=1, space="PSUM") as ps_accp,
            tc.tile_pool(name="ps_fc", bufs=2, space="PSUM") as ps_fcp,
            tc.tile_pool(name="ps_f2", bufs=4,
                         space="PSUM") as ps_f2p,
            tc.tile_pool(name="ps_small", bufs=1, space="PSUM") as ps_smallp,
            tc.tile_pool(name="dram", bufs=1, space="DRAM") as dramp,
        ):
            # ---------------- constants ----------------
            # class pattern 0..7 tiled GROUP times (bf16: fp8 memset is not
            # a valid ISA instruction), for the batched one-hot is_equal
            clsrep = constp.tile([P, GROUP, C], BF16)
            for c in range(C):
                nc.vector.memset(clsrep[:, :, c], float(c))
            cls8 = constp.tile([P, C], F32)
            for c in range(C):
                nc.vector.memset(cls8[:, c : c + 1], float(c))
            pidx_i = constp.tile([C, 1], mybir.dt.int32)
            nc.gpsimd.iota(pidx_i[:], pattern=[[0, 1]], base=0,
                           channel_multiplier=1)
            pidx = constp.tile([C, 1], F32)
            nc.vector.tensor_copy(pidx[:], pidx_i[:])
            ident8 = constp.tile([C, C], F32)
            nc.vector.tensor_scalar(
                ident8[:], cls8[0:C, :], pidx[:], None, op0=OP.is_equal
            )
            ones1 = constp.tile([1, P], BF16)   # K=1 lhsT for c2 rank-1 fill
            nc.vector.memset(ones1[:], 1.0)
            ones_col = constp.tile([P, 1], F32)  # rhs for final partition sum
            nc.vector.memset(ones_col[:], 1.0)
            ones_f8 = constp.tile([P, 1], FP8)   # rhs for the PE f2 reduce
            nc.vector.tensor_copy(ones_f8[:], ones_col[:])

            # targets, host-swizzled: tg[p, t] = target of row t*128+p
            tg = constp.tile([P, T], FP8)
            nc.sync.dma_start(tg[:], t_in.ap())

            for rep in range(reps):
                f2_all = constp.tile([P, T], F32, tag="f2all",
                                     name=f"f2all{rep}")
                if f2_mode == "off":
                    nc.vector.memset(f2_all[:], 1.0)
                acc_blk = constp.tile([P, nblk], F32, tag="accblk",
                                      name=f"accblk{rep}")
                if not (p2 and ft_loads):
                    nc.vector.memset(acc_blk[:], 0.0)
                ps_sums = ps_accp.tile([C, DW], F32, tag="ps_sums")
                npe = min(pe_blocks, nblk) if (
                    f2_mode == "batch" and p2 and ft_loads and nat_loads
                ) else 0

                # -------- pass 1 (+ early pass-2 transposed loads) --------
                fts = {}
                fts2 = {}
                psf2 = {}
                f2_emitted = set()

                def emit_ft2(b2):
                    ftA_, ftB_ = fts[b2]
                    ft2A = ft2p.tile([P, BLOCK * P], FP8, tag="f2A")
                    ft2B = ft2p.tile([P, BLOCK * P], FP8, tag="f2B")
                    if b2 >= npe - ft2_dve:
                        nc.vector.tensor_tensor(ft2A[:], ftA_[:], ftA_[:],
                                                op=OP.mult)
                        nc.vector.tensor_tensor(ft2B[:], ftB_[:], ftB_[:],
                                                op=OP.mult)
                    else:
                        nc.scalar.activation(ft2A[:], ftA_[:], AFT.Square)
                        nc.scalar.activation(ft2B[:], ftB_[:], AFT.Square)
                    fts2[b2] = (ft2A, ft2B)
                    psf2[b2] = ps_f2p.tile([P, BLOCK], F32, tag="psf2",
                                           name=f"psf2_{rep}_{b2}")

                def emit_f2_half(b2, half):
                    f2_emitted.add((b2, half))
                    ft2A, ft2B = fts2[b2]
                    ps2 = psf2[b2]
                    for qq in range(half * (BLOCK // 2),
                                    (half + 1) * (BLOCK // 2)):
                        o2 = ps2[:, qq : qq + 1]
                        nc.tensor.matmul(
                            o2, ft2A[:, qq * P : (qq + 1) * P],
                            ones_f8[:], start=True, stop=False,
                        )
                        nc.tensor.matmul(
                            o2, ft2B[:, qq * P : (qq + 1) * P],
                            ones_f8[:], start=False, stop=True,
                        )
                gpb = BLOCK // GROUP
                for g in range(G):
                    st = stagep.tile([P, GROUP, DW], FP8, tag="stage")
                    if nat_loads:
                        nc.sync.dma_start(
                            st[:],
                            f_n.ap()[:, g * GROUP * DW : (g + 1) * GROUP * DW]
                            .rearrange("p (u d) -> p u d", d=DW),
                        )
                    else:
                        nc.vector.memset(st[:, 0, 0:1], 0.0)
                    gpb_ = BLOCK // GROUP
                    pe_g = f2_mode == "batch" and g < npe * gpb_
                    npe1 = min(npe, FT_PREFETCH)
                    if g >= 3 and (g - 3) % 2 == 0 and (g - 3) // 2 < npe1:
                        emit_ft2((g - 3) // 2)
                    if g >= 4 and (g - 4) // 2 < npe1:
                        emit_f2_half((g - 4) // 2, (g - 4) % 2)
                    if f2_mode == "batch" and not pe_g:
                        sq_g = sqp.tile([P, GROUP, DW], BF16, tag="sqg")
                        sq_in = st[:].rearrange("p u d -> p (u d)")
                        sq_out = sq_g[:].rearrange("p u d -> p (u d)")
                        if g >= G - dve_sq_groups:
                            nc.vector.tensor_tensor(
                                sq_out, sq_in, sq_in, op=OP.mult
                            )
                        else:
                            nc.scalar.activation(sq_out, sq_in, AFT.Square)
                    oh_g = ohp.tile([P, GROUP, C], FP8, tag="oh")
                    tgb = (
                        tg[:, g * GROUP : (g + 1) * GROUP]
                        .rearrange("p (u o) -> p u o", o=1)
                        .broadcast_to([P, GROUP, C])
                    )
                    nc.vector.tensor_tensor(
                        oh_g[:], tgb, clsrep[:], op=OP.is_equal
                    )
                    for u in range(GROUP):
                        t = g * GROUP + u
                        if p1mm:
                            nc.tensor.matmul(
                                ps_sums[:], oh_g[:, u, :], st[:, u, :],
                                start=(t == 0), stop=(t == T - 1),
                            )
                        if f2_mode == "off":
                            pass
                        elif f2_mode == "batch" and not pe_g:
                            # f2 + 1 (ones col squared); the +1 is
                            # compensated in the c2 row of pass 2
                            tso = sqvp.tile([P, DW], BF16, tag="tso")
                            nc.vector.tensor_scalar(
                                tso[:], sq_g[:, u, :], 1.0, None,
                                op0=OP.mult, op1=OP.add,
                                accum_out=f2_all[:, t : t + 1],
                            )
                        elif f2_mode == "scalar":
                            sq = sqp.tile([P, D], F32, tag="sq")
                            nc.scalar.activation(
                                sq[:], st[:, u, 0:D], AFT.Square,
                                accum_out=f2_all[:, t : t + 1],
                            )
                    # prefetch transposed loads for the first FT_PREFETCH
                    # blocks only (later blocks would emit waits on pass-2
                    # progress into the pass-1 stream -> deadlock)
                    if ft_loads and (g + 1) % gpb == 0:
                        b = (g + 1) // gpb - 1
                        if b < min(FT_PREFETCH, nblk):
                            fts[b] = _issue_ft(nc, ftp, f_ta, f_tb, b)

                for b2 in range(min(npe, FT_PREFETCH)):
                    if b2 not in fts2:
                        emit_ft2(b2)
                    for half in range(2):
                        if (b2, half) not in f2_emitted:
                            emit_f2_half(b2, half)

                # ---------------- all-reduce ----------------
                sb_sums = smallp.tile([C, DW], F32, tag="sbs",
                                      name=f"sbs{rep}")
                if p1mm:
                    nc.vector.tensor_copy(sb_sums[:], ps_sums[:])
                else:
                    nc.vector.memset(sb_sums[:], 1.0)
                cc_in = dramp.tile([C, DW], F32, name=f"cci{rep}")
                cc_out = dramp.tile([C, DW], F32, name=f"cco{rep}")
                nc.gpsimd.dma_start(cc_in[:], sb_sums[:])
                nc.gpsimd.collective_compute(
                    "AllReduce", OP.add,
                    replica_groups=[list(range(NCORES))],
                    ins=[cc_in.opt()], outs=[cc_out.opt()],
                )
                gsums = smallp.tile([C, DW], F32, tag="gs", name=f"gs{rep}")
                nc.gpsimd.dma_start(gsums[:], cc_out[:])

                # ---------------- centers ----------------
                counts = gsums[:, D : D + 1]
                cnt1 = smallp.tile([C, 1], F32, tag="cnt", name=f"cnt{rep}")
                nc.vector.tensor_scalar_max(cnt1[:], counts, 1.0)
                recip = smallp.tile([C, 1], F32, tag="rcp", name=f"rcp{rep}")
                nc.vector.reciprocal(recip[:], cnt1[:])
                centers = smallp.tile([C, D], F32, tag="ctr", name=f"ctr{rep}")
                nc.vector.tensor_scalar(
                    centers[:], gsums[:, 0:D], recip[:], None, op0=OP.mult
                )
                csq = smallp.tile([C, D], F32, tag="csq", name=f"csq{rep}")
                nc.vector.tensor_tensor(
                    csq[:], centers[:], centers[:], op=OP.mult
                )
                c2 = smallp.tile([C, 1], F32, tag="c2_", name=f"c2_{rep}")
                nc.vector.reduce_sum(c2[:], csq[:], axis=mybir.AxisListType.X)
                emptyb = smallp.tile([C, 1], F32, tag="emp", name=f"emp{rep}")
                nc.vector.tensor_scalar(
                    emptyb[:], counts, 0.5, BIG, op0=OP.is_lt, op1=OP.mult
                )
                c2m = smallp.tile([C, 1], F32, tag="c2m", name=f"c2m{rep}")
                nc.vector.tensor_tensor(c2m[:], c2[:], emptyb[:], op=OP.add)
                c2mm1 = smallp.tile([C, 1], F32, tag="c2n", name=f"c2n{rep}")
                if f2_mode == "batch":
                    # non-PE blocks carry f2+1 (squared ones col): subtract
                    nc.vector.tensor_scalar_add(c2mm1[:], c2m[:], -1.0)
                else:
                    nc.vector.tensor_copy(c2mm1[:], c2m[:])

                # stationary weights: W[k] = -2 * centers[:, 128k:+128].T fp8
                ctb = []
                for k in range(2):
                    ps_t = ps_smallp.tile([P, C], F32, tag="ps_small")
                    nc.tensor.transpose(
                        ps_t[:], centers[:, k * P : (k + 1) * P], ident8[:]
                    )
                    w = constp.tile([P, C], FP8, tag=f"ctw{k}",
                                    name=f"ctw{rep}_{k}")
                    nc.vector.tensor_scalar_mul(w[:], ps_t[:], -2.0)
                    ctb.append(w)

                # c2 rows replicated BLOCK times on one partition: [1, 512]
                c2reps = []
                for nm, c2src in (("a", c2m), ("b", c2mm1)):
                    ps_r = ps_smallp.tile([1, C], F32, tag="ps_small")
                    nc.tensor.transpose(ps_r[:], c2src[:], ident8[:])
                    c2rep = constp.tile([1, BLOCK * C], BF16, tag=f"c2rep{nm}",
                                        name=f"c2rep{nm}{rep}")
                    nc.vector.tensor_copy(c2rep[:, 0:C], ps_r[:])
                    w_ = C
                    while w_ < BLOCK * C:
                        nc.vector.tensor_copy(c2rep[:, w_ : 2 * w_],
                                              c2rep[:, 0:w_])
                        w_ *= 2
                    c2reps.append(c2rep)

                # ---------------- pass 2 ----------------
                d2all = constp.tile([P, nblk * BLOCK], F32, tag="d2all",
                                    name=f"d2all{rep}")
                for b in range(nblk if (p2 and ft_loads) else 0):
                    ftA, ftB = fts[b]
                    ps_fc = ps_fcp.tile([P, BLOCK * C], F32, tag="ps_fc")
                    nxt = b + FT_PREFETCH
                    if nxt < nblk:
                        fts[nxt] = _issue_ft(nc, ftp, f_ta, f_tb, nxt)
                    b2n = b + 2
                    if (min(npe, FT_PREFETCH) <= b2n < npe
                            and b2n not in fts2):
                        emit_ft2(b2n)
                        emit_f2_half(b2n, 0)
                        emit_f2_half(b2n, 1)
                    # rank-1 fill: every row of the bank gets the c2 pattern
                    c2row = c2reps[0] if b < npe else c2reps[1]
                    nc.tensor.matmul(
                        ps_fc[:], ones1[:], c2row[:], start=True, stop=False
                    )
                    for q in range(BLOCK):
                        o = ps_fc[:, q * C : (q + 1) * C]
                        nc.tensor.matmul(
                            o, ftA[:, q * P : (q + 1) * P], ctb[0][:],
                            start=False, stop=False,
                        )
                        nc.tensor.matmul(
                            o, ftB[:, q * P : (q + 1) * P], ctb[1][:],
                            start=False, stop=(q == BLOCK - 1),
                        )
                    mn = distp.tile([P, BLOCK], F32, tag="mn")
                    nc.vector.tensor_reduce(
                        mn[:], ps_fc[:].rearrange("p (t c) -> p t c", c=C),
                        axis=mybir.AxisListType.X, op=OP.min,
                    )
                    d2 = d2all[:, b * BLOCK : (b + 1) * BLOCK]
                    f2src = (
                        psf2[b][:] if b < npe
                        else f2_all[:, b * BLOCK : (b + 1) * BLOCK]
                    )
                    nc.vector.tensor_tensor(d2, mn[:], f2src, op=OP.add)
                    nc.vector.tensor_scalar_max(d2, d2, 0.0)

                # ---------------- final reduce ----------------
                tot = smallp.tile([P, 1], F32, tag="tot", name=f"tot{rep}")
                if p2 and ft_loads:
                    sroot = distp.tile([P, nblk * BLOCK], BF16, tag="sroot")
                    nc.scalar.activation(
                        sroot[:], d2all[:], AFT.Sqrt, accum_out=tot[:]
                    )
                else:
                    nc.vector.reduce_sum(
                        tot[:], acc_blk[:], axis=mybir.AxisListType.X
                    )
                ps_tot = ps_smallp.tile([1, 1], F32, tag="ps_small")
                nc.tensor.matmul(
                    ps_tot[:], tot[:], ones_col[:], start=True, stop=True
                )
                res = smallp.tile([1, 1], F32, tag="res", name=f"res{rep}")
                nc.vector.tensor_copy(res[:], ps_tot[:])
                nc.sync.dma_start(out_t.ap()[rep : rep + 1, :], res[:])

    nc.compile()
    return nc


USE_BF16 = False  # legacy knob kept for the harness; fp8 path is canonical

_CACHE = {}


def _get_nc(R: int):
    if R not in _CACHE:
        _CACHE[R] = build_nc(R)
    return _CACHE[R]


def _fp8(x: np.ndarray) -> np.ndarray:
    np8 = mybir.dt.np(FP8)
    return np.clip(x, -240.0, 240.0).astype(np8)


def make_in_maps(features: np.ndarray, targets: np.ndarray,
                 ncores: int = NCORES):
    n = features.shape[0]
    r = n // ncores
    t = r // P
    in_maps = []
    for k in range(ncores):
        sl = slice(k * r, (k + 1) * r)
        fs = np.asarray(features[sl], dtype=np.float32)
        # natural, swizzled: [p, t, 0:256] = f[t*128+p], [p, t, 256] = 1.0
        nat = np.ones((P, t, DW), dtype=np.float32)
        nat[:, :, 0:D] = fs.reshape(t, P, D).transpose(1, 0, 2)
        tgt = np.ascontiguousarray(
            targets[sl].astype(np.float32).reshape(t, P).T
        )
        in_maps.append(
            {
                "features_n": _fp8(nat).reshape(P, t * DW),
                "features_ta": _fp8(np.ascontiguousarray(fs[:, 0:P].T)),
                "features_tb": _fp8(np.ascontiguousarray(fs[:, P:D].T)),
                "targets_f": _fp8(tgt),
            }
        )
    return in_maps


def kernel(features, targets, **run_kwargs):
    features = np.asarray(features)
    targets = np.asarray(targets)
    n = features.shape[0]
    r = n // NCORES
    nc = _get_nc(r)
    in_maps = make_in_maps(features, targets)
    res = bass_utils.run_bass_kernel_spmd(
        nc, in_maps, core_ids=list(range(NCORES)), **run_kwargs
    )
    total = np.float64(0.0)
    for k in range(NCORES):
        total += np.float64(res.results[k]["partial"][0, 0])
    out = np.float32(total / n)
    if run_kwargs:
        return out, res
    return out


if __name__ == "__main__":
    nc = build_nc(8192)
    print("built OK")
